# revision 1
# baseline (speedup 1.0000x reference)
"""Masked nearest-neighbor (AnchorTs2Vec e_an) Trainium2 kernel.

Problem: for e_actv [8192, 256] f32 and host ids [8192], compute
    d2[i,j] = |e_i|^2 + |e_j|^2 - 2 e_i.e_j   (masked BIG where host_i==host_j, incl. diag)
    idx[i]  = argmin_j d2[i,j]   (first index on ties, matching jnp.argmin)
    e_an    = e_actv[idx]
Returns (e_actv, e_ap, e_an) like the reference.

Distribution: rows sharded across 8 NeuronCores (1024 rows/core); the
column operand (all 8192 embeddings) is replicated to every core, so no
collective is needed.

Device computation per core (per 128-row tile):
  - One fused matmul chain computes a NEGATED masked distance surrogate
        nval[i,j] = 2*G_ij - sq_j - 32768*[host_i==host_j]
    in PSUM via an extended contraction: fp16 hi split Eh=fp16(e) plus
    the Eh.El cross term (El = e-Eh, exact power-of-two +-64 balancing),
    G ~ Eh.Eh + Eh.El, |error| <~ 4e-3; plus 3 fp16 splits of sq_j and
    a -32768*onehot64(host) mask block (5 K-chunks total -- sized so
    the warm PE stream ~ matches the DVE span, keeping the PE dense and
    HAM-warm). sq_i omitted (row constant), sqrt omitted (monotone).
  - ACT copies PSUM -> SBUF as fp16 (frees PSUM banks).
  - DVE max8 per column group + combine: top1 = -min, top2 = runner-up.
  - DVE is_equal(val, top1) -> uint8 one-hot mask, DMA'd to the host.
Host: idx = mask.argmax(1) (exact first-index); rows whose device
margin (top1-top2) cannot certify the true argmin (fp16 copy quant
0.25 + model err) are recomputed exactly in fp32 numpy (<~100 rows);
final gather e_actv[idx].
"""

import numpy as np

import concourse.tile as tile
from concourse import bacc, mybir
from concourse.bass_utils import run_bass_kernel_spmd

N, D, H = 8192, 256, 64
N_CORES = 8
RPC = N // N_CORES          # rows per core
P = 128                     # partitions
RT = RPC // P               # row tiles per core (8)
TS = 512                    # matmul free-dim (one PSUM bank)
GW = 2048                   # column group width (4 PSUM banks)
NG = N // GW                # column groups (4)
NCHUNK = 3                  # Eh0 Eh1 extras
CTW = NCHUNK * GW
BIGM = 32768.0
SCALE = np.float32(64.0)
RESCUE_THR = 0.8            # device margin below which the host recomputes

f16 = np.float16

_compiled = None


def _build():
    nc = bacc.Bacc("TRN2", target_bir_lowering=False, debug=False,
                   num_devices=N_CORES)
    b_in = nc.dram_tensor("b_in", [NG, P, CTW], mybir.dt.float16,
                          kind="ExternalInput").ap()
    a_in = nc.dram_tensor("a_in", [RT, P, NCHUNK * P], mybir.dt.float16,
                          kind="ExternalInput").ap()
    out_mask = nc.dram_tensor("out_mask", [RPC, N], mybir.dt.float16,
                              kind="ExternalOutput").ap()

    with tile.TileContext(nc) as tc:
        with tc.tile_pool(name="bp", bufs=1) as bp, \
             tc.tile_pool(name="apool", bufs=4) as apool, \
             tc.tile_pool(name="vp", bufs=3) as vp, \
             tc.tile_pool(name="mp", bufs=2) as mp, \
             tc.tile_pool(name="small", bufs=4) as small, \
             tc.tile_pool(name="psum", bufs=2, space="PSUM") as pp:
            def load_a(rt):
                at = apool.tile([P, NCHUNK * P], mybir.dt.float16, tag="a")
                nc.sync.dma_start(at[:], a_in[rt])
                return at

            atiles = [load_a(0), load_a(1)]
            btile = bp.tile([P, NG * CTW], mybir.dt.float16, tag="b")
            for c in range(NCHUNK):
                nc.sync.dma_start(btile[:, c * GW:(c + 1) * GW],
                                  b_in[0, :, c * GW:(c + 1) * GW])
            for g in range(1, NG):
                nc.sync.dma_start(btile[:, g * CTW:(g + 1) * CTW], b_in[g])

            for rt in range(RT):
                at = atiles[rt]
                if rt + 2 < RT:
                    atiles.append(load_a(rt + 2))
                v = vp.tile([P, N], mybir.dt.float16, tag="val")
                acc = mp.tile([P, GW], mybir.dt.float16, tag="acc")
                for g in range(NG):
                    ps = pp.tile([P, GW], mybir.dt.float32, tag="ps")
                    for s in range(GW // TS):
                        for c in range(NCHUNK):
                            lhsT = at[:, c * P:(c + 1) * P]
                            boff = g * CTW + c * GW + s * TS
                            rhs = btile[:, boff:boff + TS]
                            nc.tensor.matmul(
                                ps[:, s * TS:(s + 1) * TS], lhsT, rhs,
                                start=(c == 0), stop=(c == NCHUNK - 1))
                    nc.scalar.activation(v[:, g * GW:(g + 1) * GW], ps[:],
                                         mybir.ActivationFunctionType.Copy)
                    if g == 1:
                        nc.vector.tensor_tensor(acc[:], v[:, 0:GW],
                                                v[:, GW:2 * GW],
                                                op=mybir.AluOpType.max)
                    elif g > 1:
                        acc2 = mp.tile([P, GW], mybir.dt.float16, tag="acc")
                        nc.vector.tensor_tensor(acc2[:], acc[:],
                                                v[:, g * GW:(g + 1) * GW],
                                                op=mybir.AluOpType.max)
                        acc = acc2

                t1 = small.tile([P, 1], mybir.dt.float32, tag="t1")
                nc.vector.tensor_reduce(t1[:], acc[:], axis=mybir.AxisListType.X,
                                        op=mybir.AluOpType.max)
                thr = small.tile([P, 1], mybir.dt.float32, tag="thr")
                nc.vector.tensor_scalar(thr[:], t1[:], -RESCUE_THR, None,
                                        op0=mybir.AluOpType.add)
                mask = mp.tile([P, N], mybir.dt.float16, tag="mask")
                nc.vector.tensor_scalar(mask[:], v[:], thr[:, 0:1], None,
                                        op0=mybir.AluOpType.is_ge)
                r0 = rt * P
                nc.sync.dma_start(out_mask[r0:r0 + P, :], mask[:])

    nc.compile()
    return nc


def _prep_inputs(e_actv: np.ndarray, host: np.ndarray):
    e = np.ascontiguousarray(np.asarray(e_actv, dtype=np.float32))
    hostv = np.asarray(host).astype(np.int64)

    eh = e.astype(f16)
    ehf = eh.astype(np.float32)
    elf = e - ehf

    chunks_b = [eh]
    chunks_a = [(2.0 * ehf).astype(f16)]

    sq = (e.astype(np.float64) * e.astype(np.float64)).sum(1)
    s1 = sq.astype(np.float32).astype(f16)
    r1 = (sq - s1.astype(np.float64)).astype(np.float32)
    s2 = r1.astype(f16)
    s3 = (r1 - s2.astype(np.float32)).astype(f16)

    onehot = np.zeros((N, H), dtype=np.float32)
    onehot[np.arange(N), hostv] = 1.0

    bstack = np.zeros((NCHUNK, P, N), dtype=f16)
    for kk, cb in enumerate(chunks_b):
        cbT = np.ascontiguousarray(cb.T)
        bstack[2 * kk] = cbT[:P]
        bstack[2 * kk + 1] = cbT[P:]
    bx = np.zeros((P, N), dtype=np.float32)
    bx[0] = -s1.astype(np.float32)
    bx[1] = -s2.astype(np.float32)
    bx[2] = -s3.astype(np.float32)
    bx[3:3 + H] = onehot.T * np.float32(-BIGM)
    bstack[2] = bx.astype(f16)
    b_all = np.ascontiguousarray(
        bstack.reshape(NCHUNK, P, NG, GW).transpose(2, 1, 0, 3)
    ).reshape(NG, P, CTW)

    a_maps = []
    for core in range(N_CORES):
        a_all = np.zeros((RT, P, NCHUNK * P), dtype=f16)
        for rt in range(RT):
            r0 = core * RPC + rt * P
            rows = slice(r0, r0 + P)
            for kk, ca in enumerate(chunks_a):
                caT = np.ascontiguousarray(ca[rows].T)
                a_all[rt, :, (2 * kk) * P:(2 * kk + 1) * P] = caT[:P]
                a_all[rt, :, (2 * kk + 1) * P:(2 * kk + 2) * P] = caT[P:]
            ax = np.zeros((P, P), dtype=np.float32)
            ax[0:3, :] = 1.0
            ax[3:3 + H, :] = onehot[rows].T
            a_all[rt, :, 2 * P:3 * P] = ax.astype(f16)
        a_maps.append({"b_in": b_all, "a_in": a_all})
    return a_maps


def _run(in_maps, trace=False, **kw):
    global _compiled
    if _compiled is None:
        _compiled = _build()
    return run_bass_kernel_spmd(_compiled, in_maps, list(range(N_CORES)),
                                trace=trace, **kw)


def _exact_rows(e, hostv, rows):
    """Exact fp32 masked argmin for the given rows (reference semantics)."""
    sq = (e * e).sum(1)
    G = e[rows] @ e.T
    d2 = sq[rows][:, None] + sq[None, :] - 2.0 * G
    d2 = np.where(hostv[rows][:, None] == hostv[None, :], np.float32(1e30), d2)
    return d2.argmin(1)


def kernel(e_actv, e_ap, host):
    e = np.ascontiguousarray(np.asarray(e_actv, dtype=np.float32))
    hostv = np.asarray(host).astype(np.int64)
    in_maps = _prep_inputs(e, hostv)
    res = _run(in_maps)

    masks = np.concatenate([res.results[c]["out_mask"] for c in range(N_CORES)])
    marked = masks > 0
    idx = marked.argmax(axis=1)
    rescue = np.where(marked.sum(axis=1) != 1)[0]
    if len(rescue):
        idx[rescue] = _exact_rows(e, hostv, rescue)

    e_an = np.asarray(e_actv)[idx]
    return (np.asarray(e_actv), np.asarray(e_ap), e_an)

